# revision 6
# baseline (speedup 1.0000x reference)
"""DiKT (DKVMN-style knowledge tracing) Trainium2 kernel.

Self-contained: builds a Bass/Tile program, shards batch over 8 NeuronCores
(pure data parallel, 16 batch rows per core), runs via run_bass_kernel_spmd.

Algorithm per core (B_loc=16, V=128, C=64, S=128 steps):
  Both value memories (right/wrong) live as ONE SBUF tensor m[v=128, col=2048]
  with col = r*64 + c, r = mem*16 + b.  Per step:
     m' = m * (1 - e x w) + a x w
  The rank-1 outer products are built on the TensorEngine using a
  block-diagonal (negated) w matrix:  wdiag[r, col] = -w[r,c] * (r block),
     E' = e_t^T . wdiag   (= -e x w),   S = 1 + E'   (ScalarE bias)
     A  = (-a_t)^T . wdiag (= +a x w)
  DVE:  u = m * S ;  m = u + A   (fp16 SBUF, 2x mode)

All per-step e, a, w are precomputed up front from embedding gathers
(indirect DMA) + small matmuls; wdiag for all steps is staged in DRAM and
streamed during the loop.
"""

import numpy as np

import concourse.mybir as mybir
from concourse import bass, bacc, tile
from concourse.bass_utils import run_bass_kernel_spmd

F16 = mybir.dt.float16
F32 = mybir.dt.float32
I32 = mybir.dt.int32
ALU = mybir.AluOpType
ACT = mybir.ActivationFunctionType
AX = mybir.AxisListType

# model dims
KD = 128      # KEY_DIM
VD = 128      # VALUE_DIM
SD = 128      # SUMMARY_DIM
Q = 10000     # QUESTION_NUM
C = 64        # CONCEPT_NUM
B = 128       # full batch
S = 128       # seq len
NCORE = 8
BL = B // NCORE          # 16 batch rows per core
NR = 2 * BL              # 32 rows per step (right+wrong)
COLS = NR * C            # 2048 memory columns per core
NG = (S * NR) // 128     # 32 gather chunks of 128 rows
WD_STEP = NR * COLS // NR  # elements per (step,row) = 2048
WD_T = NR * COLS // NR * NR  # 65536 elements per step block (32*2048)

DEBUG = False


def _build_program():
    nc = bacc.Bacc(trn_type="TRN2", target_bir_lowering=False, num_devices=NCORE)

    # ---- DRAM inputs ----
    i_emb = nc.dram_tensor("i_emb", [2 * Q + 1, VD], F32, kind="ExternalInput")
    q_emb = nc.dram_tensor("q_emb", [Q + 1, KD], F32, kind="ExternalInput")
    idx_i = nc.dram_tensor("idx_i", [128, NG], I32, kind="ExternalInput")
    idx_q = nc.dram_tensor("idx_q", [128, NG], I32, kind="ExternalInput")
    idx_t = nc.dram_tensor("idx_t", [BL, 1], I32, kind="ExternalInput")

    erase_Wt = nc.dram_tensor("erase_Wt", [VD, VD], F16, kind="ExternalInput")
    add_Wt = nc.dram_tensor("add_Wt", [VD, VD], F16, kind="ExternalInput")
    key_Wt = nc.dram_tensor("key_Wt", [KD, C], F16, kind="ExternalInput")
    erase_b_row = nc.dram_tensor("erase_b_row", [1, VD], F16, kind="ExternalInput")
    add_b_row = nc.dram_tensor("add_b_row", [1, VD], F16, kind="ExternalInput")
    rsum_Wt0 = nc.dram_tensor("rsum_Wt0", [VD, SD], F16, kind="ExternalInput")
    rsum_Wt1 = nc.dram_tensor("rsum_Wt1", [KD, SD], F16, kind="ExternalInput")
    wsum_Wt0 = nc.dram_tensor("wsum_Wt0", [VD, SD], F16, kind="ExternalInput")
    wsum_Wt1 = nc.dram_tensor("wsum_Wt1", [KD, SD], F16, kind="ExternalInput")
    rsum_b_col = nc.dram_tensor("rsum_b_col", [SD, 1], F32, kind="ExternalInput")
    wsum_b_col = nc.dram_tensor("wsum_b_col", [SD, 1], F32, kind="ExternalInput")
    succ_Wt = nc.dram_tensor("succ_Wt", [SD, 1], F16, kind="ExternalInput")
    fail_Wt = nc.dram_tensor("fail_Wt", [SD, 1], F16, kind="ExternalInput")
    diff_Wt = nc.dram_tensor("diff_Wt", [KD, 1], F16, kind="ExternalInput")
    succ_b = nc.dram_tensor("succ_b", [1, 1], F32, kind="ExternalInput")
    fail_b = nc.dram_tensor("fail_b", [1, 1], F32, kind="ExternalInput")
    diff_b = nc.dram_tensor("diff_b", [1, 1], F32, kind="ExternalInput")
    rmem0 = nc.dram_tensor("rmem0", [VD, C], F16, kind="ExternalInput")
    wmem0 = nc.dram_tensor("wmem0", [VD, C], F16, kind="ExternalInput")
    ones_row = nc.dram_tensor("ones_row", [1, 128], F16, kind="ExternalInput")
    ones_col32 = nc.dram_tensor("ones_col32", [128, 1], F32, kind="ExternalInput")
    id128 = nc.dram_tensor("id128", [128, 128], F16, kind="ExternalInput")
    right_full = nc.dram_tensor("right_full", [B, S], I32, kind="ExternalInput")
    wrong_full = nc.dram_tensor("wrong_full", [B, S], I32, kind="ExternalInput")

    out_d = nc.dram_tensor("out", [BL, 1], F32, kind="ExternalOutput")
    if DEBUG:
        dbg_m = nc.dram_tensor("dbg_m", [VD, COLS], F16, kind="ExternalOutput")
        dbg_e = nc.dram_tensor("dbg_e", [128, S], F16, kind="ExternalOutput")
        dbg_na = nc.dram_tensor("dbg_na", [128, S], F16, kind="ExternalOutput")
        dbg_rr = nc.dram_tensor("dbg_rr", [VD, NR], F32, kind="ExternalOutput")
        dbg_wd = nc.dram_tensor("dbg_wd", [NR, COLS], F16, kind="ExternalOutput")

    # wdiag for every step, flat fp16: element (t, r, col) at t*65536 + r*2048 + col
    wd_dram = nc.dram_tensor("wd_dram", [S * NR * COLS], F16)

    # ---- persistent SBUF ----
    sb = lambda name, shape, dt: nc.alloc_sbuf_tensor(name, shape, dt)
    m_sb = sb("m_sb", [VD, COLS], F16)
    vecT = sb("vecT", [128, NG * 128], F16)   # i_emb rows, transposed, fp16
    qT = sb("qT", [128, NG * 128], F16)
    e_all = sb("e_all", [128, NG * 128], F16)  # sigmoid(erase)
    na_all = sb("na_all", [128, NG * 128], F16)  # -tanh(add)
    w_eWt = sb("w_eWt", [VD, VD], F16)
    w_aWt = sb("w_aWt", [VD, VD], F16)
    w_kWt = sb("w_kWt", [KD, C], F16)
    w_eb = sb("w_eb", [1, VD], F16)
    w_ab = sb("w_ab", [1, VD], F16)
    w_ones = sb("w_ones", [1, 128], F16)
    w_ones_c32 = sb("w_ones_c32", [128, 1], F32)
    w_id = sb("w_id", [128, 128], F16)
    idx_i_sb = sb("idx_i_sb", [128, NG], I32)
    idx_q_sb = sb("idx_q_sb", [128, NG], I32)
    idx_t_sb = sb("idx_t_sb", [BL, 1], I32)
    w_rs0 = sb("w_rs0", [VD, SD], F16)
    w_rs1 = sb("w_rs1", [KD, SD], F16)
    w_ws0 = sb("w_ws0", [VD, SD], F16)
    w_ws1 = sb("w_ws1", [KD, SD], F16)
    w_rsb = sb("w_rsb", [SD, 1], F32)
    w_wsb = sb("w_wsb", [SD, 1], F32)
    w_succ = sb("w_succ", [SD, 1], F16)
    w_fail = sb("w_fail", [SD, 1], F16)
    w_diff = sb("w_diff", [KD, 1], F16)
    w_sb_b = sb("w_sb_b", [1, 3], F32)  # succ_b, fail_b, diff_b columns 0..2
    zeros2k = sb("zeros2k", [128, COLS], F16)

    with tile.TileContext(nc) as tc:
        with tc.tile_pool(name="sbp", bufs=3) as sbp:
            # ---------- load constants ----------
            for dst, src in [
                (w_eWt, erase_Wt), (w_aWt, add_Wt), (w_kWt, key_Wt),
                (w_eb, erase_b_row), (w_ab, add_b_row), (w_ones, ones_row),
                (w_ones_c32, ones_col32), (w_id, id128),
                (idx_i_sb, idx_i), (idx_q_sb, idx_q), (idx_t_sb, idx_t),
                (w_rs0, rsum_Wt0), (w_rs1, rsum_Wt1),
                (w_ws0, wsum_Wt0), (w_ws1, wsum_Wt1),
                (w_rsb, rsum_b_col), (w_wsb, wsum_b_col),
                (w_succ, succ_Wt), (w_fail, fail_Wt), (w_diff, diff_Wt),
            ]:
                nc.sync.dma_start(out=dst[:, :], in_=src[:, :])
            nc.sync.dma_start(out=w_sb_b[:, 0:1], in_=succ_b[:, :])
            nc.sync.dma_start(out=w_sb_b[:, 1:2], in_=fail_b[:, :])
            nc.sync.dma_start(out=w_sb_b[:, 2:3], in_=diff_b[:, :])

            # zero-fill wd_dram (16 MiB fp16) from a zeroed sbuf tile
            nc.gpsimd.memset(zeros2k[:, :], 0.0)
            for g in range(NG):
                nc.scalar.dma_start(
                    out=bass.AP(wd_dram, g * 4 * WD_T, [[2048, 128], [1, 2048]]),
                    in_=zeros2k[:, :],
                )

            # init m: broadcast mem inits over the 16 batch blocks
            rmem_t = sbp.tile([VD, C], F16, tag="memi")
            nc.sync.dma_start(out=rmem_t[:, :], in_=rmem0[:, :])
            wmem_t = sbp.tile([VD, C], F16, tag="memi2")
            nc.sync.dma_start(out=wmem_t[:, :], in_=wmem0[:, :])
            for r in range(NR):
                srct = rmem_t if r < BL else wmem_t
                nc.vector.tensor_copy(m_sb[:, r * C:(r + 1) * C], srct[:, :])

            # ---------- gathers + transposes ----------
            with tc.tile_pool(name="pst", bufs=2, space="PSUM") as psp:
                for g in range(NG):
                    lo = g * 128
                    # i-table chunk
                    gi32 = sbp.tile([128, VD], F32, tag="gi32")
                    nc.gpsimd.indirect_dma_start(
                        out=gi32[:, :], out_offset=None,
                        in_=i_emb[:, :],
                        in_offset=bass.IndirectOffsetOnAxis(
                            ap=idx_i_sb[:, g:g + 1], axis=0),
                    )
                    gi16 = sbp.tile([128, VD], F16, tag="gi16")
                    nc.vector.tensor_copy(gi16[:, :], gi32[:, :])
                    tps = psp.tile([128, 128], F16, tag="tp")
                    nc.tensor.transpose(tps[:, :], gi16[:, :], w_id[:, :])
                    nc.vector.tensor_copy(vecT[:, lo:lo + 128], tps[:, :])
                    # q-table chunk
                    gq32 = sbp.tile([128, KD], F32, tag="gq32")
                    nc.gpsimd.indirect_dma_start(
                        out=gq32[:, :], out_offset=None,
                        in_=q_emb[:, :],
                        in_offset=bass.IndirectOffsetOnAxis(
                            ap=idx_q_sb[:, g:g + 1], axis=0),
                    )
                    gq16 = sbp.tile([128, KD], F16, tag="gq16")
                    nc.vector.tensor_copy(gq16[:, :], gq32[:, :])
                    tps2 = psp.tile([128, 128], F16, tag="tp2")
                    nc.tensor.transpose(tps2[:, :], gq16[:, :], w_id[:, :])
                    nc.vector.tensor_copy(qT[:, lo:lo + 128], tps2[:, :])

            # ---------- e/a/w precompute ----------
            # grouped by ACT table set: all Sigmoid, then all Tanh, then Exp
            with tc.tile_pool(name="psz", bufs=2, space="PSUM") as psp:
                for g in range(NG):
                    lo = g * 128
                    # e = sigmoid(vec @ erase_W.T + erase_b)
                    eps = psp.tile([128, VD], F32, tag="eps")
                    nc.tensor.matmul(eps[:, :], vecT[:, lo:lo + 128], w_eWt[:, :],
                                     start=True, stop=False)
                    nc.tensor.matmul(eps[:, :], w_ones[:, :], w_eb[:, :],
                                     start=False, stop=True)
                    nc.scalar.activation(e_all[:, lo:lo + 128], eps[:, :], ACT.Sigmoid)
                for g in range(NG):
                    lo = g * 128
                    # na = -tanh(vec @ add_W.T + add_b)
                    aps = psp.tile([128, VD], F32, tag="aps")
                    nc.tensor.matmul(aps[:, :], vecT[:, lo:lo + 128], w_aWt[:, :],
                                     start=True, stop=False)
                    nc.tensor.matmul(aps[:, :], w_ones[:, :], w_ab[:, :],
                                     start=False, stop=True)
                    nc.scalar.activation(na_all[:, lo:lo + 128], aps[:, :], ACT.Tanh,
                                         scale=-1.0)
                for g in range(NG):
                    lo = g * 128
                    # w = softmax(qv @ key_W.T); store NEGATED into wdiag dram
                    zps = psp.tile([128, C], F32, tag="zps")
                    nc.tensor.matmul(zps[:, :], qT[:, lo:lo + 128], w_kWt[:, :],
                                     start=True, stop=True)
                    mx = sbp.tile([128, 1], F32, tag="mx")
                    nc.vector.tensor_reduce(mx[:, :], zps[:, :], AX.X, ALU.max)
                    nmx = sbp.tile([128, 1], F32, tag="nmx")
                    nc.vector.tensor_scalar_mul(nmx[:, :], mx[:, :], -1.0)
                    wex = sbp.tile([128, C], F32, tag="wex")
                    nc.scalar.activation(wex[:, :], zps[:, :], ACT.Exp,
                                         bias=nmx[:, :], scale=1.0)
                    sm = sbp.tile([128, 1], F32, tag="sm")
                    nc.vector.tensor_reduce(sm[:, :], wex[:, :], AX.X, ALU.add)
                    rc = sbp.tile([128, 1], F32, tag="rc")
                    nc.vector.reciprocal(rc[:, :], sm[:, :])
                    wng = sbp.tile([128, C], F16, tag="wng")
                    nc.vector.tensor_scalar(wng[:, :], wex[:, :], rc[:, :], -1.0,
                                            ALU.mult, ALU.mult)
                    # scatter the 4 steps' diagonal blocks
                    nc.sync.dma_start(
                        out=bass.AP(wd_dram, 4 * g * WD_T,
                                    [[WD_T, 4], [COLS + C, NR], [1, C]]),
                        in_=wng[:, :],
                    )

            # ---------- the recurrence ----------
            with tc.tile_pool(name="psl", bufs=1, space="PSUM") as psl, \
                 tc.tile_pool(name="wdp", bufs=2) as wdp, \
                 tc.tile_pool(name="sul", bufs=2) as sul:
                for t in range(S):
                    g, s = divmod(t, 4)
                    if s == 0:
                        wd4 = wdp.tile([128, COLS], F16, tag="wd4")
                        nc.sync.dma_start(
                            out=wd4[:, :],
                            in_=bass.AP(wd_dram, 4 * g * WD_T,
                                        [[2048, 128], [1, 2048]]),
                        )
                    p0 = 32 * s
                    eL = e_all[p0:p0 + 32, g * 128:(g + 1) * 128]
                    aL = na_all[p0:p0 + 32, g * 128:(g + 1) * 128]
                    wdr = wd4[p0:p0 + 32, :]
                    Eps = psl.tile([128, COLS], F32, tag="Eps")
                    Aps = psl.tile([128, COLS], F32, tag="Aps")
                    for k in range(4):
                        nc.tensor.matmul(Eps[:, 512 * k:512 * (k + 1)], eL,
                                         wdr[:, 512 * k:512 * (k + 1)],
                                         start=True, stop=True,
                                         tile_position=(p0, 0))
                    for k in range(4):
                        nc.tensor.matmul(Aps[:, 512 * k:512 * (k + 1)], aL,
                                         wdr[:, 512 * k:512 * (k + 1)],
                                         start=True, stop=True,
                                         tile_position=(p0, 0))
                    S_t = sul.tile([128, COLS], F16, tag="S_t")
                    nc.scalar.activation(S_t[:, :], Eps[:, :], ACT.Identity, bias=1.0)
                    A_t = sul.tile([128, COLS], F16, tag="A_t")
                    nc.scalar.activation(A_t[:, :], Aps[:, :], ACT.Copy, bias=0.0)
                    u_t = sul.tile([128, COLS], F16, tag="u_t")
                    nc.vector.tensor_tensor(u_t[:, :], m_sb[:, :], S_t[:, :], ALU.mult)
                    nc.vector.tensor_tensor(m_sb[:, :], u_t[:, :], A_t[:, :], ALU.add)

            # ---------- readout + head ----------
            with tc.tile_pool(name="psr", bufs=1, space="PSUM") as psr, \
                 tc.tile_pool(name="sbr", bufs=1) as sbr:
                # target question embedding, transposed
                qv32 = sbr.tile([BL, KD], F32, tag="qv32")
                nc.gpsimd.indirect_dma_start(
                    out=qv32[:, :], out_offset=None,
                    in_=q_emb[:, :],
                    in_offset=bass.IndirectOffsetOnAxis(ap=idx_t_sb[:, 0:1], axis=0),
                )
                qv16 = sbr.tile([BL, KD], F16, tag="qv16")
                nc.vector.tensor_copy(qv16[:, :], qv32[:, :])
                qvT_ps = psr.tile([KD, BL], F16, tag="psmall")
                nc.tensor.transpose(qvT_ps[:, :], qv16[:, :], w_id[:BL, :BL])
                qvT = sbr.tile([KD, BL], F16, tag="qvT")
                nc.vector.tensor_copy(qvT[:, :], qvT_ps[:, :])

                # wt = softmax(qv @ key_W.T)
                zt = psr.tile([BL, C], F32, tag="psmall")
                nc.tensor.matmul(zt[:, :], qvT[:, :], w_kWt[:, :], start=True, stop=True)
                mxt = sbr.tile([BL, 1], F32, tag="mxt")
                nc.vector.tensor_reduce(mxt[:, :], zt[:, :], AX.X, ALU.max)
                nmxt = sbr.tile([BL, 1], F32, tag="nmxt")
                nc.vector.tensor_scalar_mul(nmxt[:, :], mxt[:, :], -1.0)
                wext = sbr.tile([BL, C], F32, tag="wext")
                nc.scalar.activation(wext[:, :], zt[:, :], ACT.Exp, bias=nmxt[:, :])
                smt = sbr.tile([BL, 1], F32, tag="smt")
                nc.vector.tensor_reduce(smt[:, :], wext[:, :], AX.X, ALU.add)
                rct = sbr.tile([BL, 1], F32, tag="rct")
                nc.vector.reciprocal(rct[:, :], smt[:, :])
                wt16 = sbr.tile([BL, C], F16, tag="wt16")
                nc.vector.tensor_scalar_mul(wt16[:, :], wext[:, :], rct[:, :])
                # flatten to (1, 2048): [right blocks | wrong blocks], both = wt
                wtf = sbr.tile([1, COLS], F16, tag="wtf")
                nc.gpsimd.dma_start(out=wtf[0:1, 0:BL * C], in_=wt16[:, :])
                nc.gpsimd.dma_start(out=wtf[0:1, BL * C:COLS], in_=wt16[:, :])
                # broadcast over partitions via K=1 matmul
                wb_ps = psr.tile([128, COLS], F32, tag="wb_ps")
                for k in range(4):
                    nc.tensor.matmul(wb_ps[:, 512 * k:512 * (k + 1)], w_ones[:, :],
                                     wtf[:, 512 * k:512 * (k + 1)],
                                     start=True, stop=True)
                wb = sbr.tile([128, COLS], F16, tag="wb")
                nc.scalar.activation(wb[:, :], wb_ps[:, :], ACT.Copy, bias=0.0)
                u2 = sbr.tile([128, COLS], F16, tag="u2")
                nc.vector.tensor_tensor(u2[:, :], m_sb[:, :], wb[:, :], ALU.mult)
                rr = sbr.tile([VD, NR], F32, tag="rr")
                nc.vector.tensor_reduce(
                    rr[:, :], u2[:].rearrange("p (r c) -> p r c", c=C), AX.X, ALU.add)
                rr16 = sbr.tile([VD, NR], F16, tag="rr16")
                nc.vector.tensor_copy(rr16[:, :], rr[:, :])

                # r_sum / w_sum: (SD, BL)
                rs_ps = psr.tile([SD, BL], F32, tag="psmall")
                nc.tensor.matmul(rs_ps[:, :], w_rs0[:, :], rr16[:, 0:BL],
                                 start=True, stop=False)
                nc.tensor.matmul(rs_ps[:, :], w_rs1[:, :], qvT[:, :],
                                 start=False, stop=True)
                rsum = sbr.tile([SD, BL], F16, tag="rsum")
                nc.scalar.activation(rsum[:, :], rs_ps[:, :], ACT.Tanh,
                                     bias=w_rsb[:, :])
                ws_ps = psr.tile([SD, BL], F32, tag="psmall")
                nc.tensor.matmul(ws_ps[:, :], w_ws0[:, :], rr16[:, BL:NR],
                                 start=True, stop=False)
                nc.tensor.matmul(ws_ps[:, :], w_ws1[:, :], qvT[:, :],
                                 start=False, stop=True)
                wsum = sbr.tile([SD, BL], F16, tag="wsum")
                nc.scalar.activation(wsum[:, :], ws_ps[:, :], ACT.Tanh,
                                     bias=w_wsb[:, :])

                # success/failure/difficulty levels: (1, BL)
                lv_ps = psr.tile([1, BL], F32, tag="psmall")
                succ = sbr.tile([1, BL], F32, tag="succ")
                nc.tensor.matmul(lv_ps[:, :], w_succ[:, :], rsum[:, :],
                                 start=True, stop=True)
                nc.scalar.activation(succ[:, :], lv_ps[:, :], ACT.Tanh,
                                     bias=w_sb_b[:, 0:1])
                lv_ps2 = psr.tile([1, BL], F32, tag="psmall")
                fail = sbr.tile([1, BL], F32, tag="fail")
                nc.tensor.matmul(lv_ps2[:, :], w_fail[:, :], wsum[:, :],
                                 start=True, stop=True)
                nc.scalar.activation(fail[:, :], lv_ps2[:, :], ACT.Tanh,
                                     bias=w_sb_b[:, 1:2])
                lv_ps3 = psr.tile([1, BL], F32, tag="psmall")
                diff = sbr.tile([1, BL], F32, tag="diff")
                nc.tensor.matmul(lv_ps3[:, :], w_diff[:, :], qvT[:, :],
                                 start=True, stop=True)
                nc.scalar.activation(diff[:, :], lv_ps3[:, :], ACT.Tanh,
                                     bias=w_sb_b[:, 2:3])

                # global success/failure counts (use FULL inputs, same all cores)
                sigs = sbr.tile([1, 2], F32, tag="sigs")
                for ci, full in enumerate([right_full, wrong_full]):
                    fin = sbr.tile([B, S], I32, tag="fin")
                    nc.sync.dma_start(out=fin[:, :], in_=full[:, :])
                    ff = sbr.tile([B, S], F32, tag="ff")
                    nc.vector.tensor_copy(ff[:, :], fin[:, :])
                    fc = sbr.tile([B, S], F32, tag="fc")
                    nc.vector.tensor_scalar(fc[:, :], ff[:, :], 1.0, None, ALU.min)
                    cs = sbr.tile([B, 1], F32, tag="cs")
                    nc.vector.tensor_reduce(cs[:, :], fc[:, :], AX.X, ALU.add)
                    cnt_ps = psr.tile([1, 1], F32, tag="psmall")
                    nc.tensor.matmul(cnt_ps[:, :], cs[:, :], w_ones_c32[:, :],
                                     start=True, stop=True)
                    nc.scalar.activation(sigs[:, ci:ci + 1], cnt_ps[:, :], ACT.Sigmoid)

                # out = succ*sig(sc) + fail*sig(fc) - 2*diff
                t1 = sbr.tile([1, BL], F32, tag="t1")
                nc.vector.tensor_scalar_mul(t1[:, :], succ[:, :], sigs[:, 0:1])
                t2 = sbr.tile([1, BL], F32, tag="t2")
                nc.vector.tensor_scalar_mul(t2[:, :], fail[:, :], sigs[:, 1:2])
                t3 = sbr.tile([1, BL], F32, tag="t3")
                nc.vector.tensor_scalar_mul(t3[:, :], diff[:, :], -2.0)
                o1 = sbr.tile([1, BL], F32, tag="o1")
                nc.vector.tensor_tensor(o1[:, :], t1[:, :], t2[:, :], ALU.add)
                o2 = sbr.tile([1, BL], F32, tag="o2")
                nc.vector.tensor_tensor(o2[:, :], o1[:, :], t3[:, :], ALU.add)
                nc.sync.dma_start(out=out_d[:, :], in_=o2[:, :])

                if DEBUG:
                    nc.sync.dma_start(out=dbg_m[:, :], in_=m_sb[:, :])
                    nc.sync.dma_start(out=dbg_e[:, :], in_=e_all[:, 0:S])
                    nc.sync.dma_start(out=dbg_na[:, :], in_=na_all[:, 0:S])
                    nc.sync.dma_start(out=dbg_rr[:, :], in_=rr[:, :])
                    nc.sync.dma_start(
                        out=dbg_wd[:, :],
                        in_=bass.AP(wd_dram, 0, [[2048, 32], [1, 2048]]))

    nc.compile()
    return nc


_PROGRAM = None


def _get_program():
    global _PROGRAM
    if _PROGRAM is None:
        _PROGRAM = _build_program()
    return _PROGRAM


def _host_inputs(inputs):
    """Build the per-core in_maps from the full problem inputs."""
    f16 = np.float16
    f32 = np.float32
    ri = np.asarray(inputs["right_input"]).astype(np.int64)
    wi = np.asarray(inputs["wrong_input"]).astype(np.int64)
    tg = np.asarray(inputs["target_id"]).astype(np.int64)
    q_emb = np.asarray(inputs["q_emb"], dtype=f32)
    i_emb = np.asarray(inputs["i_emb"], dtype=f32)

    def W(name):
        return np.asarray(inputs[name], dtype=f32)

    common = {
        "i_emb": i_emb,
        "q_emb": q_emb,
        "erase_Wt": np.ascontiguousarray(W("erase_W").T).astype(f16),
        "add_Wt": np.ascontiguousarray(W("add_W").T).astype(f16),
        "key_Wt": np.ascontiguousarray(W("key_W").T).astype(f16),
        "erase_b_row": W("erase_b").reshape(1, -1).astype(f16),
        "add_b_row": W("add_b").reshape(1, -1).astype(f16),
        "rsum_Wt0": np.ascontiguousarray(W("rsum_W")[:, :VD].T).astype(f16),
        "rsum_Wt1": np.ascontiguousarray(W("rsum_W")[:, VD:].T).astype(f16),
        "wsum_Wt0": np.ascontiguousarray(W("wsum_W")[:, :VD].T).astype(f16),
        "wsum_Wt1": np.ascontiguousarray(W("wsum_W")[:, VD:].T).astype(f16),
        "rsum_b_col": W("rsum_b").reshape(-1, 1).astype(f32),
        "wsum_b_col": W("wsum_b").reshape(-1, 1).astype(f32),
        "succ_Wt": np.ascontiguousarray(W("succ_W").T).astype(f16),
        "fail_Wt": np.ascontiguousarray(W("fail_W").T).astype(f16),
        "diff_Wt": np.ascontiguousarray(W("diff_W").T).astype(f16),
        "succ_b": W("succ_b").reshape(1, 1).astype(f32),
        "fail_b": W("fail_b").reshape(1, 1).astype(f32),
        "diff_b": W("diff_b").reshape(1, 1).astype(f32),
        "rmem0": W("right_mem_init").astype(f16),
        "wmem0": W("wrong_mem_init").astype(f16),
        "ones_row": np.ones((1, 128), dtype=f16),
        "ones_col32": np.ones((128, 1), dtype=f32),
        "id128": np.eye(128, dtype=f16),
        "right_full": ri.astype(np.int32),
        "wrong_full": wi.astype(np.int32),
    }

    in_maps = []
    for core in range(NCORE):
        rows = slice(core * BL, (core + 1) * BL)
        # inter ids per (t, r): r<BL -> right, else wrong
        inter = np.empty((S, NR), dtype=np.int64)
        inter[:, :BL] = ri[rows].T
        inter[:, BL:] = wi[rows].T
        qid = inter - Q * (inter > Q)
        flat_i = inter.reshape(-1)
        flat_q = qid.reshape(-1)
        idx_i = flat_i.reshape(NG, 128).T.astype(np.int32)
        idx_q = flat_q.reshape(NG, 128).T.astype(np.int32)
        idx_t = tg[rows].reshape(BL, 1).astype(np.int32)
        in_maps.append({**common, "idx_i": np.ascontiguousarray(idx_i),
                        "idx_q": np.ascontiguousarray(idx_q),
                        "idx_t": idx_t})
    return in_maps


def run_spmd(inputs, trace=False):
    nc = _get_program()
    in_maps = _host_inputs(inputs)
    res = run_bass_kernel_spmd(nc, in_maps, core_ids=list(range(NCORE)),
                               trace=trace)
    out = np.concatenate([res.results[i]["out"] for i in range(NCORE)], axis=0)
    return out.astype(np.float32), res


def kernel(**inputs):
    out, _ = run_spmd(inputs, trace=False)
    return out
